# revision 17
# baseline (speedup 1.0000x reference)
"""AttnLSTMDecoder Trainium2 (Bass/Tile) kernel.

Strictly sequential batch-1 decode: T=1024 steps, L=4096 attention memory,
STATE=ATTN=EMB=100, VOCAB=128.  Per the sharding hint there is exactly one
sequence, so the whole recurrence runs on ONE NeuronCore (cross-core
AllReduce floor ~10us/step would dominate any L-sharding).

Device-side step (all SBUF-resident, one activation table set
`exp_and_others` = {tanh, exp}, loaded once):

  s       = W1b @ [h; c]                       2 matmuls  [100,1] psum
  a1      = tanh(preT + s)                     ACT, bias=per-partition s,
                                               preT = (im@W1a.T+b1).T  [100,4096]
  scores  = w2.T @ a1 (+b2 later)              8 matmuls N=512 -> psum [1,512]
  scoresT [128,32] <- DMA partition-reshape    (l = 32p+j block map)
  e       = exp(scoresT + b2)                  ACT
  uctx    = sum_j e[:,j].T @ imB[:,j,:]        32 acc-matmuls -> [1,202] psum
            (imB cols = [im0:100, 1, im100:200, 1] so cols 100/201 give esum)
  ctxn    = uctx * (1/esum)                    DVE reciprocal + tensor_scalar
  ctxA/B  = PE-transpose of ctxn halves        [101,1] each (aug row == 1)
  gates   = 16 matmuls [101,100].T @ [101,1]   bias folded in emb-chunk row 100
  th      = tanh(0.5*gates)                    sigma(x)=0.5*(1+tanh(x/2));
                                               g-gate weights pre-doubled
  c2      = 0.5*((c+g) + th_f*c + th_i*g)
  h2      = 0.5*(th_o*tanh(c2) + tanh(c2))
  logits  = [h2;1].T @ [W_out.T; b_out]        -> psum [1,128]
  E_t     = accum_out of ACT exp(logits)       -> esums[t]
  lc_t    = [h2;1].T @ Wsel[:,t]               = logits[char_t] (+bias)

Host: result = sum_t log(E_t) - lc_t.
"""

import os
import sys
from contextlib import ExitStack

import numpy as np

sys.path.insert(0, "/opt/trn_rl_repo")

VOCAB, S, A, E, L, T, EOS = 128, 100, 100, 100, 4096, 1024, 0


# ---------------------------------------------------------------- host prep
def _prep(inputs):
    f32 = np.float32
    im = np.asarray(inputs["input_mat"], f32)[0]
    out_ids = np.asarray(inputs["out_ids"])
    emb = np.asarray(inputs["emb"], f32)
    W1 = np.asarray(inputs["W1"], f32); b1 = np.asarray(inputs["b1"], f32)
    W2 = np.asarray(inputs["W2"], f32); b2 = np.asarray(inputs["b2"], f32)
    W_ih = np.asarray(inputs["W_ih"], f32); W_hh = np.asarray(inputs["W_hh"], f32)
    b_ih = np.asarray(inputs["b_ih"], f32); b_hh = np.asarray(inputs["b_hh"], f32)
    W_out = np.asarray(inputs["W_out"], f32); b_out = np.asarray(inputs["b_out"], f32)

    W1a, W1b = W1[:, : 2 * S], W1[:, 2 * S :]
    d = {}
    d["preT"] = np.ascontiguousarray((im @ W1a.T + b1).T)          # [A, L]
    imB = np.empty((128, 32, 202), f32)
    imr = im.reshape(32, 128, 200).transpose(1, 0, 2)   # imB[p,j] = im[128j+p]
    imB[:, :, 0:100] = imr[:, :, 0:100]
    imB[:, :, 100] = 1.0
    imB[:, :, 101:201] = imr[:, :, 100:200]
    imB[:, :, 201] = 1.0
    d["imB"] = imB.reshape(128, 32 * 202)
    d["w2c"] = np.ascontiguousarray(W2[0].reshape(A, 1))
    b2s = float(b2[0])
    W1bT = np.empty((100, 200), f32)
    W1bT[:, 0:100] = W1b[:, 0:100].T
    W1bT[:, 100:200] = W1b[:, 100:200].T
    d["W1bT"] = W1bT
    rows = {0: slice(0, 100), 1: slice(100, 200), 2: slice(300, 400), 3: slice(200, 300)}
    bsum = b_ih + b_hh
    WgT = np.zeros((101, 16 * 100), f32)
    for g in range(4):
        mul = 2.0 if g == 3 else 1.0
        for q in range(4):
            blk = np.zeros((101, 100), f32)
            if q < 3:
                blk[0:100] = (W_ih[rows[g], 100 * q : 100 * q + 100] * mul).T
                if q == 2:
                    blk[100] = bsum[rows[g]] * mul
            else:
                blk[0:100] = (W_hh[rows[g], :] * mul).T
            WgT[:, (g * 4 + q) * 100 : (g * 4 + q + 1) * 100] = blk
    d["WgT"] = WgT
    WoutTb = np.empty((101, VOCAB), f32)
    WoutTb[0:100] = W_out.T
    WoutTb[100] = b_out
    d["WoutTb"] = WoutTb
    embT = np.empty((101, T), f32)
    prev = np.concatenate([[EOS], out_ids[:-1]]).astype(np.int64)
    embT[0:100] = emb[prev].T
    embT[100] = 1.0
    d["embT"] = embT
    Wsel = np.empty((101, T), f32)
    Wsel[0:100] = W_out[out_ids].T
    Wsel[100] = b_out[out_ids]
    d["Wsel"] = Wsel
    d["ident"] = np.eye(128, dtype=f32)

    # pack everything into one [128, NC] constant block (single DMA load)
    blocks = [
        ("preT", d["preT"]), ("imB", d["imB"]), ("W1bT", d["W1bT"]),
        ("WgT", d["WgT"]), ("embT", d["embT"]), ("Wselw", d["Wsel"][0:100]),
        ("Woutw", d["WoutTb"][0:100]), ("w2c", d["w2c"]),
        ("Woutb", d["WoutTb"][100:101]), ("Wselb", d["Wsel"][100:101]),
        ("ident", d["ident"]),
    ]
    nc_total = sum(b.shape[1] for _, b in blocks)
    CONST = np.zeros((128, nc_total), f32)
    offs = {}
    o = 0
    for name, b in blocks:
        CONST[0 : b.shape[0], o : o + b.shape[1]] = b
        offs[name] = o
        o += b.shape[1]
    offs["ncols"] = nc_total
    return {"CONST": CONST, "offs": offs}, b2s


# ---------------------------------------------------------------- bass build
def build_program(T_steps=T, b2s=0.0, offs=None):
    import concourse.bass as bass
    import concourse.tile as tile
    from concourse import bacc, mybir

    AF = mybir.ActivationFunctionType
    DT = mybir.dt.float32
    nc = bacc.Bacc("TRN2", target_bir_lowering=False, debug=False)

    ncols = offs["ncols"]
    CONST_d = nc.declare_dram_parameter("CONST", [128, ncols], DT, isOutput=False)
    esums_d = nc.declare_dram_parameter("esums", [1, T], DT, isOutput=True)
    lcs_d = nc.declare_dram_parameter("lcs", [1, T], DT, isOutput=True)

    with ExitStack() as ctx:
        tc = ctx.enter_context(tile.TileContext(nc))
        const = ctx.enter_context(tc.tile_pool(name="const", bufs=1))

        CONST = const.tile([128, offs["ncols"]], DT, tag="CONST")
        nc.gpsimd.dma_start(CONST[:], CONST_d[:])

        def blk(name, nrow, ncol):
            o = offs[name]
            return CONST[0:nrow, o : o + ncol]

        preT = blk("preT", A, L)
        imB = blk("imB", 128, 32 * 202).rearrange("p (j n) -> p j n", n=202)
        w2c = blk("w2c", A, 1)
        W1bT = blk("W1bT", 100, 200)
        WgT = blk("WgT", 101, 1600)
        Woutw = blk("Woutw", 100, VOCAB)
        Woutb = blk("Woutb", 1, VOCAB)
        embT = blk("embT", 101, T)
        Wselw = blk("Wselw", 100, T)
        Wselb = blk("Wselb", 1, T)
        ident = blk("ident", 128, 128)

        esums_sb = const.tile([1, T], DT)
        lcs_sb = const.tile([1, T], DT)
        if T_steps == 0:
            nc.vector.memset(esums_sb[:], 1.0)
            nc.vector.memset(lcs_sb[:], 0.0)
        b2t = const.tile([128, 1], DT)
        nc.vector.memset(b2t[:], float(b2s))
        onecell = const.tile([1, 1], DT)
        nc.vector.memset(onecell[:], 1.0)
        BF = mybir.dt.bfloat16
        w2cb = const.tile([A, 1], BF, tag="w2cb")
        nc.vector.tensor_copy(w2cb[:], w2c[:])
        imBb = const.tile([128, 32 * 202], BF, tag="imBb")
        nc.vector.tensor_copy(imBb[:], blk("imB", 128, 32 * 202))
        imBb = imBb.rearrange("p (j n) -> p j n", n=202)
        Woutwb = const.tile([100, VOCAB], BF, tag="Woutwb")
        nc.vector.tensor_copy(Woutwb[:], Woutw[:])
        Woutbb = const.tile([1, VOCAB], BF, tag="Woutbb")
        nc.vector.tensor_copy(Woutbb[:], Woutb[:])
        Wselwb = const.tile([100, T], BF, tag="Wselwb")
        nc.vector.tensor_copy(Wselwb[:], Wselw[:])
        Wselbb = const.tile([1, T], BF, tag="Wselbb")
        nc.vector.tensor_copy(Wselbb[:], Wselb[:])
        onecellb = const.tile([1, 1], BF, tag="onecellb")
        nc.vector.tensor_copy(onecellb[:], onecell[:])

        st = ctx.enter_context(tc.tile_pool(name="state", bufs=3))
        sb = ctx.enter_context(tc.tile_pool(name="work", bufs=2))
        tmp = ctx.enter_context(tc.tile_pool(name="tmp", bufs=4))
        a1p = ctx.enter_context(tc.tile_pool(name="a1", bufs=2))
        psS = ctx.enter_context(tc.tile_pool(name="psS", bufs=1, space="PSUM"))
        psSc = ctx.enter_context(tc.tile_pool(name="psSc", bufs=2, space="PSUM"))
        psU = ctx.enter_context(tc.tile_pool(name="psU", bufs=1, space="PSUM"))
        psT = ctx.enter_context(tc.tile_pool(name="psT", bufs=1, space="PSUM"))
        psG = ctx.enter_context(tc.tile_pool(name="psG", bufs=1, space="PSUM"))
        psL = ctx.enter_context(tc.tile_pool(name="psL", bufs=1, space="PSUM"))

        h = st.tile([100, 1], DT, tag="h")
        nc.vector.memset(h[:], 0.0)
        c = st.tile([100, 1], DT, tag="c")
        nc.vector.memset(c[:], 0.0)

        NCH = 2            # tanh chunks
        CW = L // NCH      # chunk width

        for t in range(T_steps):
            # ---- s = W1b @ [h; c]
            s_ps = psS.tile([A, 1], DT, tag="s")
            nc.tensor.matmul(s_ps[:], W1bT[:, 0:100], h[:], start=True, stop=False)
            nc.tensor.matmul(s_ps[:], W1bT[:, 100:200], c[:], start=False, stop=True)
            s_sb = tmp.tile([A, 1], DT, tag="s_sb")
            nc.vector.tensor_copy(s_sb[:], s_ps[:])

            # ---- a1 = tanh(preT + s); scoresT[p,j] = w2.a1[:,128j+p] direct in psum
            scT_ps = psSc.tile([128, 32], DT, tag="scT")
            for ch in range(NCH):
                a1 = a1p.tile([A, CW], BF, tag="a1")
                nc.scalar.activation(a1[:], preT[:, ch * CW : (ch + 1) * CW], AF.Tanh,
                                     bias=s_sb[:, 0:1])
                for kk in range(CW // 128):
                    j = ch * (CW // 128) + kk
                    nc.tensor.matmul(scT_ps[:, j : j + 1],
                                     a1[:, kk * 128 : kk * 128 + 128], w2cb[:],
                                     start=True, stop=True)
            e = sb.tile([128, 32], BF, tag="e")
            nc.scalar.activation(e[:], scT_ps[:], AF.Exp, bias=b2t[:, 0:1])

            # ---- uctx = sum_j e[:,j].T @ imB[:,j,:]   (cols 100/201 = esum)
            uctx_ps = psU.tile([1, 202], DT, tag="uctx")
            for j in range(32):
                nc.tensor.matmul(uctx_ps[:], e[:, j : j + 1], imBb[:, j, :],
                                 start=(j == 0), stop=(j == 31))
            uctx = tmp.tile([1, 202], DT, tag="uctx_sb")
            nc.vector.tensor_copy(uctx[:], uctx_ps[:])
            inv = tmp.tile([1, 1], DT, tag="inv")
            nc.vector.reciprocal(inv[:], uctx[0:1, 100:101])
            ctxn = tmp.tile([1, 202], DT, tag="ctxn")
            nc.vector.tensor_scalar_mul(ctxn[:], uctx[:], inv[0:1, 0:1])

            ctx_ps = psT.tile([101, 2], DT, tag="ctxT")
            nc.tensor.transpose(ctx_ps[:, 0:1], ctxn[0:1, 0:101], ident[0:1, 0:1])
            nc.tensor.transpose(ctx_ps[:, 1:2], ctxn[0:1, 101:202], ident[0:1, 0:1])
            ctx_cols = tmp.tile([101, 2], DT, tag="ctx_cols")
            nc.vector.tensor_copy(ctx_cols[:], ctx_ps[:])

            # ---- gates: 16 matmuls [101,100].T @ [101,1]
            g_ps = psG.tile([100, 4], DT, tag="g")
            vecq = [ctx_cols[:, 0:1], ctx_cols[:, 1:2], embT[:, t : t + 1], h[:]]
            for g in range(4):
                for q in range(4):
                    wgt = WgT[:, (g * 4 + q) * 100 : (g * 4 + q + 1) * 100]
                    if q == 3:
                        wgt = WgT[0:100, (g * 4 + q) * 100 : (g * 4 + q + 1) * 100]
                    nc.tensor.matmul(g_ps[:, g : g + 1], wgt,
                                     vecq[q], start=(q == 0), stop=(q == 3))
            th = tmp.tile([100, 4], DT, tag="th")
            nc.scalar.activation(th[:], g_ps[:], AF.Tanh, scale=0.5)

            # ---- c2 = 0.5*((c+g) + th_f*c + th_i*g) ; h2 = 0.5*(th_o*thc2 + thc2)
            t1 = tmp.tile([100, 1], DT, tag="t1")
            nc.vector.tensor_add(t1[:], c[:], th[:, 3:4])
            t2 = tmp.tile([100, 1], DT, tag="t2")
            nc.vector.tensor_mul(t2[:], th[:, 1:2], c[:])
            t3 = tmp.tile([100, 1], DT, tag="t3")
            nc.vector.tensor_mul(t3[:], th[:, 0:1], th[:, 3:4])
            nc.vector.tensor_add(t1[:], t1[:], t2[:])
            nc.vector.tensor_add(t1[:], t1[:], t3[:])
            c2 = st.tile([100, 1], DT, tag="c")
            nc.vector.tensor_scalar_mul(c2[:], t1[:], 0.5)
            thc2 = tmp.tile([100, 1], DT, tag="thc2")
            nc.scalar.activation(thc2[:], c2[:], AF.Tanh)
            t4 = tmp.tile([100, 1], DT, tag="t4")
            nc.vector.tensor_mul(t4[:], th[:, 2:3], thc2[:])
            nc.vector.tensor_add(t4[:], t4[:], thc2[:])
            h2 = st.tile([100, 1], DT, tag="h")
            nc.vector.tensor_scalar_mul(h2[:], t4[:], 0.5)
            h2b = tmp.tile([100, 1], BF, tag="h2b")
            nc.vector.tensor_copy(h2b[:], h2[:])

            # ---- logits / E_t / lc_t
            log_ps = psL.tile([1, 132], DT, tag="log")
            nc.tensor.matmul(log_ps[0:1, 0:VOCAB], h2b[:], Woutwb[:], start=True, stop=False)
            nc.tensor.matmul(log_ps[0:1, 0:VOCAB], onecellb[:], Woutbb[:], start=False, stop=True)
            nc.tensor.matmul(log_ps[0:1, 128:129], h2b[:], Wselwb[:, t : t + 1],
                             start=True, stop=False)
            nc.tensor.matmul(log_ps[0:1, 128:129], onecellb[:], Wselbb[0:1, t : t + 1],
                             start=False, stop=True)
            elog = tmp.tile([1, VOCAB], DT, tag="elog")
            nc.scalar.activation(elog[:], log_ps[0:1, 0:VOCAB], AF.Exp,
                                 accum_out=esums_sb[0:1, t : t + 1])
            nc.vector.tensor_copy(lcs_sb[0:1, t : t + 1], log_ps[0:1, 128:129])

            h, c = h2, c2

        nc.gpsimd.dma_start(esums_d[:], esums_sb[:])
        nc.gpsimd.dma_start(lcs_d[:], lcs_sb[:])
    nc.compile()
    return nc


_CACHE = {}


def _run_device(d, b2s, T_steps=T, trace=False):
    from concourse.bass_utils import run_bass_kernel_spmd

    key = (T_steps,)
    if key not in _CACHE:
        _CACHE[key] = build_program(T_steps, b2s, d["offs"])
    nc = _CACHE[key]
    in_map = {"CONST": d["CONST"]}
    res = run_bass_kernel_spmd(nc, [in_map], [0], trace=trace)
    out = res.results[0]
    return out["esums"][0], out["lcs"][0], res


def kernel(input_mat, out_ids, emb, W1, b1, W2, b2, W_ih, W_hh, b_ih, b_hh,
           W_out, b_out):
    inputs = dict(input_mat=input_mat, out_ids=out_ids, emb=emb, W1=W1, b1=b1,
                  W2=W2, b2=b2, W_ih=W_ih, W_hh=W_hh, b_ih=b_ih, b_hh=b_hh,
                  W_out=W_out, b_out=b_out)
    d, b2s = _prep(inputs)
    esums, lcs, _ = _run_device(d, b2s)
    total = np.sum(np.log(esums.astype(np.float64)) - lcs.astype(np.float64))
    return np.float32(total)
